# revision 48
# baseline (speedup 1.0000x reference)
import numpy as np
import ml_dtypes

# Problem constants (hardcoded; kernel.py must be self-contained)
N, D, T, K, P = 4000, 256, 52, 20, 100
M = 8            # cores
NS = N // M      # 500 patients per core
KP = 32          # K padded to 32 so each t-group stays inside one partition tile
NBLK = (T * KP) // 128   # 13 blocks of 128 (t,k) rows
DC = 2           # d-chunks of 128
# main-loop t-groups per chunk: 26 groups of 2 (PSUM banks:
# pi4 2 banks x3 bufs + 2 ce banks = 8 of 8); bufs=3 gives the PE a deeper
# in-order queue so it stays ramped
GRPS = [(2 * i, 2) for i in range(26)]
NG = len(GRPS)

BF16 = ml_dtypes.bfloat16


def _make_kernel_mat(length_scale):
    t = np.arange(T, dtype=np.float32)
    sq = (t[None, :] - t[:, None]) ** 2
    Kmat = np.exp(-0.5 * sq / np.float32(length_scale) ** 2).astype(np.float32)
    jitter = 1e-4
    eye = np.eye(T, dtype=np.float32)
    while True:
        if np.linalg.cond(Kmat + jitter * eye) < 1e4:
            break
        jitter *= 2
        if jitter > 0.1:
            break
    return (Kmat + jitter * eye).astype(np.float32)


_KINV_LAM = np.linalg.inv(_make_kernel_mat(T / 4).astype(np.float64))
_KINV_PHI = np.linalg.inv(_make_kernel_mat(T / 3).astype(np.float64))

_COMPILED = {}


def _build_nc():
    import os
    import concourse.bass as bass
    import concourse.mybir as mybir
    from concourse import bacc, tile

    use_rcp = os.environ.get("KRCP", "1") == "1"
    use_sig = os.environ.get("KSIG", "1") == "1"
    use_paged = os.environ.get("KPAGED", "0") == "1"

    fp32 = mybir.dt.float32
    bf16 = mybir.dt.bfloat16
    Alu = mybir.AluOpType
    Act = mybir.ActivationFunctionType

    nc = bacc.Bacc(None, target_bir_lowering=False)

    # ---- DRAM inputs (host-prepacked layouts) ----
    lam32_d = nc.dram_tensor("lam32", [128, NBLK * NS], bf16, kind="ExternalInput")
    phi32_d = nc.dram_tensor("phi32", [128, NBLK * D], bf16, kind="ExternalInput")
    efT_d = nc.dram_tensor("efT", [128, DC * NS], bf16, kind="ExternalInput")
    yeT_d = nc.dram_tensor("yeT", [128, DC * NS], bf16, kind="ExternalInput")
    # lam rows interleaved with one-hole-per-k for the device-written mean
    lamg_d = nc.dram_tensor("lamg", [125, 4 * K * (T + 1)], bf16,
                            kind="ExternalInput")
    gtg_d = nc.dram_tensor("gtg", [P, NS + K], bf16, kind="ExternalInput")
    # [phi_row | lp_row] pairs for the fused phi gram
    phig2_d = nc.dram_tensor("phig2", [128, 40 * 2 * T], bf16,
                             kind="ExternalInput")
    # identity for the PE event-fold
    id_d = nc.dram_tensor("idmat", [128, 128], bf16, kind="ExternalInput")
    if not use_paged:
        # (t <= e) mask, t-major per chunk, padded to T+1 slices (slice T=0)
        am_d = nc.dram_tensor("amask", [128, DC * (T + 1) * NS], bf16,
                              kind="ExternalInput")

    # ---- DRAM outputs ----
    o_dacc = nc.dram_tensor("o_dacc", [128, 112], fp32, kind="ExternalOutput")
    o_glam = nc.dram_tensor("o_glam", [T + 1, T + 1], fp32, kind="ExternalOutput")
    o_gphi = nc.dram_tensor("o_gphi", [2 * T, 2 * T], fp32, kind="ExternalOutput")

    with tile.TileContext(nc) as tc:
        with (
            tc.tile_pool(name="res", bufs=1) as res,
            tc.tile_pool(name="scr", bufs=4) as scr,
            tc.tile_pool(name="cpx", bufs=2) as cpx,
        ):
            theta = res.tile([128, NBLK * NS], bf16, tag="theta")
            phibar = res.tile([128, NBLK * D], bf16, tag="phibar")
            ones32 = res.tile([128, KP], bf16, tag="ones32")
            nc.vector.memset(ones32[:], 1.0)
            dacc = res.tile([128, 112], fp32, tag="dacc")
            nc.vector.memset(dacc[:], 0.0)

            with (
                tc.tile_pool(name="setup", bufs=1) as setup,
                tc.tile_pool(name="soft", bufs=1) as soft,
            ):
                # critical-path DMAs first: lambda + phi feed the softmax /
                # pi-matmul chain that gates the whole main loop
                lam32 = soft.tile([128, NBLK * NS], bf16, tag="lam32")
                nc.sync.dma_start(lam32[:], lam32_d[:])
                phi32 = setup.tile([128, NBLK * D], bf16, tag="phi32")
                nc.sync.dma_start(phi32[:], phi32_d[:])
                # chunk-0 mask prefetch
                h1 = 27 * NS
                TP1 = T + 1
                if not use_paged:
                    amc0 = res.tile([128, h1], bf16, tag="amc", bufs=2,
                                    name="amc0")
                    nc.sync.dma_start(amc0[:], am_d[:, 0:h1])
                    amc20 = res.tile([128, 27 * NS], bf16, tag="amc2",
                                     bufs=2, name="amc20")
                    nc.sync.dma_start(amc20[:], am_d[:, 26 * NS:TP1 * NS])
                efT = res.tile([128, DC * NS], bf16, tag="efT")
                nc.sync.dma_start(efT[:], efT_d[:])
                yeT = res.tile([128, DC * NS], bf16, tag="yeT")
                nc.sync.dma_start(yeT[:], yeT_d[:])
                idI = res.tile([128, 128], bf16, tag="idI")
                nc.sync.dma_start(idI[:], id_d[:])

                # e32 = exp(lambda) in quarters: R matmuls of early blocks
                # start as soon as their slice is ready
                e32 = soft.tile([128, NBLK * NS], bf16, tag="e32")
                EQ = (3, 4, 3, 3)
                q0 = 0
                for nq in EQ:
                    nc.scalar.activation(
                        e32[:, q0 * NS:(q0 + nq) * NS],
                        lam32[:, q0 * NS:(q0 + nq) * NS], Act.Exp)
                    q0 += nq
                # phibar = 1 - sigmoid(phi) = sigmoid(-phi); chunk-0 d-cols
                # first (they gate the first pi matmuls)
                phv = phibar[:, :].rearrange("p (b d) -> p b d", d=D)
                piv = phi32[:, :].rearrange("p (b d) -> p b d", d=D)
                for cc in range(DC):
                    dsl = slice(cc * 128, (cc + 1) * 128)
                    if use_sig:
                        nc.scalar.activation(phv[:, :, dsl], piv[:, :, dsl],
                                             Act.Sigmoid, scale=-1.0)
                    else:
                        nc.scalar.activation(phv[:, :, dsl], piv[:, :, dsl],
                                             Act.Sigmoid)
                if not use_sig:
                    nc.vector.tensor_scalar(out=phibar[:], in0=phibar[:],
                                            scalar1=-1.0, scalar2=1.0,
                                            op0=Alu.mult, op1=Alu.add)

                # ===== theta = softmax(lambda): e * recip(R) — BEFORE the
                # GP grams so the PE unblocks the main loop asap =====
                rinv16 = soft.tile([128, NBLK * NS], bf16, tag="rinv16")
                with tc.tile_pool(name="rrp", bufs=4,
                                  space=bass.MemorySpace.PSUM) as rrp:
                    for blk in range(NBLK):
                        rrep = rrp.tile([128, 512], fp32, tag="rrep")
                        for pg in range(4):
                            nc.tensor.matmul(
                                rrep[pg * 32:(pg + 1) * 32, 0:NS],
                                ones32[pg * 32:(pg + 1) * 32, :],
                                e32[pg * 32:pg * 32 + KP,
                                    blk * NS:(blk + 1) * NS],
                                skip_group_check=True,
                                tile_position=(pg * 32, pg * 32))
                        if use_rcp:
                            rfp = scr.tile([128, NS], fp32, tag="rfp")
                            nc.vector.reciprocal_approx_fast(
                                rfp[:], rrep[:, 0:NS])
                        else:
                            rsb = scr.tile([128, NS], fp32, tag="rsb")
                            nc.scalar.activation(rsb[:], rrep[:, 0:NS],
                                                 Act.Copy)
                            with nc.allow_low_precision("softmax recip bf16"):
                                nc.vector.reciprocal(
                                    rinv16[:, blk * NS:(blk + 1) * NS],
                                    rsb[:])
                        # per-blk mult so pi matmuls of early groups can
                        # start before the whole softmax finishes
                        nc.vector.tensor_tensor(
                            out=theta[:, blk * NS:(blk + 1) * NS],
                            in0=e32[:, blk * NS:(blk + 1) * NS],
                            in1=(rfp[:] if use_rcp else
                                 rinv16[:, blk * NS:(blk + 1) * NS]),
                            op=Alu.mult)

            # ===== GP phase: mean, then fused grams (PE work that is
            # off the critical path; its tiles reuse the softmax space) =====
            with tc.tile_pool(name="gpsu", bufs=1) as setup:
                lamg = setup.tile([125, 4 * K * (T + 1)], bf16, tag="lamg")
                nc.sync.dma_start(lamg[:], lamg_d[:])
                gtg = setup.tile([P, NS + K], bf16, tag="gtg")
                nc.sync.dma_start(gtg[:], gtg_d[:])
                phig2 = setup.tile([128, 40 * 2 * T], bf16, tag="phig2")
                nc.sync.dma_start(phig2[:], phig2_d[:])
                with tc.tile_pool(name="gps", bufs=1,
                                  space=bass.MemorySpace.PSUM) as gps:
                    mean_ps = gps.tile([125, 512], fp32, tag="mean_ps")
                    for b in range(4):
                        nc.tensor.matmul(mean_ps[:, 0:K],
                                         gtg[:, b * 125:(b + 1) * 125],
                                         gtg[:, NS:NS + K])
                        # write mean into the per-(b,k) hole at slot offset 52
                        hole = lamg[:, :].rearrange(
                            "p (s w) -> p s w", w=T + 1)[:, b * K:(b + 1) * K,
                                                         T:T + 1]
                        nc.vector.tensor_copy(hole,
                                              mean_ps[:, 0:K].unsqueeze(2))

                    glam_ps = gps.tile([T + 1, 512], fp32, tag="glam_ps")
                    nmm = 4 * K
                    for i in range(nmm):
                        v = lamg[:, i * (T + 1):(i + 1) * (T + 1)]
                        nc.tensor.matmul(glam_ps[:, 0:T + 1], v, v,
                                         start=(i == 0), stop=(i == nmm - 1),
                                         skip_group_check=True)
                    gphi_ps = gps.tile([2 * T, 512], fp32, tag="gphi_ps")
                    for i in range(40):
                        v = phig2[:, i * 2 * T:(i + 1) * 2 * T]
                        nc.tensor.matmul(gphi_ps[:, 0:2 * T], v, v,
                                         start=(i == 0), stop=(i == 39),
                                         skip_group_check=True)
                    gout = setup.tile([2 * T, 2 * T + T + 1], fp32, tag="gout")
                    nc.vector.tensor_copy(gout[0:T + 1, 0:T + 1],
                                          glam_ps[0:T + 1, 0:T + 1])
                    nc.vector.tensor_copy(gout[:, T + 1:T + 1 + 2 * T],
                                          gphi_ps[:, 0:2 * T])
                    nc.sync.dma_start(o_glam[:], gout[0:T + 1, 0:T + 1])
                    nc.sync.dma_start(o_gphi[:], gout[:, T + 1:T + 1 + 2 * T])



            # ===== data-loss main loop =====
            # Per chunk: matmul pi4 = 1-pi, scalar-evict Ln -> L1f, then two
            # 2x TT mult passes against the (t<=e) mask and its t+1-shifted
            # view; PE identity-folds both products over t into two PSUM
            # banks: Ale = sum_{t<=e} L1 and Alt = sum_{t<e} L1 (per
            # element). S1 = reduce(Ale); ce = Ale - Alt = L1 at event.
            TP1 = T + 1
            with tc.tile_pool(name="big", bufs=1) as big:
                ceC = [res.tile([128, NS], bf16, tag=f"ce{c}",
                                name=f"ce{c}") for c in range(DC)]
                for c in range(DC):
                    efc = efT[:, c * NS:(c + 1) * NS]
                    if not use_paged:
                        # halves split at the t=27 group boundary; +1 slice
                        # overlap so shifted (w) views stay in-tile.
                        # chunk 0 was prefetched during setup.
                        h1 = 27 * NS
                        if c == 0:
                            amc, amc2 = amc0, amc20
                        else:
                            amc = res.tile([128, h1], bf16, tag="amc",
                                           bufs=2, name="amc1")
                            amc2 = res.tile([128, 27 * NS], bf16,
                                            tag="amc2", bufs=2, name="amc21")
                            nc.sync.dma_start(
                                amc[:],
                                am_d[:, c * TP1 * NS:c * TP1 * NS + h1])
                            nc.sync.dma_start(
                                amc2[:],
                                am_d[:, c * TP1 * NS + 26 * NS:
                                     (c + 1) * TP1 * NS])

                    with (
                        tc.tile_pool(name="pi4p", bufs=3,
                                     space=bass.MemorySpace.PSUM) as pi4p,
                        tc.tile_pool(name="cep", bufs=1,
                                     space=bass.MemorySpace.PSUM) as cep,
                        tc.tile_pool(name="vwp", bufs=3) as vwp,
                        tc.tile_pool(name="l1p", bufs=3) as l1p,
                    ):
                        ale_ps = cep.tile([128, 512], fp32, tag="ale")
                        alt_ps = cep.tile([128, 512], fp32, tag="alt")

                        def emit_pi(t0, nt):
                            pi4 = pi4p.tile([128, 2 * 512], fp32, tag="pi4")
                            for j in range(nt):
                                t = t0 + j
                                blk, prow = t // 4, 32 * (t % 4)
                                nc.tensor.matmul(
                                    pi4[:, j * 512:j * 512 + NS],
                                    phibar[prow:prow + KP,
                                           blk * D + c * 128:
                                           blk * D + c * 128 + 128],
                                    theta[prow:prow + KP,
                                          blk * NS:(blk + 1) * NS],
                                    skip_group_check=True,
                                    tile_position=(prow, 0))
                            return pi4

                        def emit_mults(t0, nt, pi4):
                            pi4v = pi4[:, :].rearrange(
                                "p (t q) -> p t q", q=512)[:, 0:nt, 0:NS]
                            l1g = l1p.tile([128, 2 * NS], bf16, tag="l1g")
                            nc.scalar.activation(
                                l1g[:, 0:nt * NS].rearrange(
                                    "p (t n) -> p t n", t=nt),
                                pi4v, Act.Ln)
                            vt = vwp.tile([128, 2 * NS], bf16, tag="v")
                            wt = vwp.tile([128, 2 * NS], bf16, tag="w")
                            if use_paged:
                                l1v = l1g[:, 0:nt * NS].rearrange(
                                    "p (t n) -> p t n", t=nt)
                                mo = efc.unsqueeze(1).broadcast_to(
                                    [128, nt, NS])
                                nc.vector.tensor_paged_mask(
                                    out=vt[:, 0:nt * NS].rearrange(
                                        "p (t n) -> p t n", t=nt),
                                    in_=l1v, partition_indices=float(t0 - 1),
                                    partition_step=1.0, mask_offsets=mo)
                                nc.vector.tensor_paged_mask(
                                    out=wt[:, 0:nt * NS].rearrange(
                                        "p (t n) -> p t n", t=nt),
                                    in_=l1v, partition_indices=float(t0),
                                    partition_step=1.0, mask_offsets=mo)
                            else:
                                if t0 >= 26:
                                    amt, aoff = amc2, (t0 - 26) * NS
                                else:
                                    amt, aoff = amc, t0 * NS
                                nc.vector.tensor_tensor(
                                    out=vt[:, 0:nt * NS],
                                    in0=amt[:, aoff:aoff + nt * NS],
                                    in1=l1g[:, 0:nt * NS], op=Alu.mult)
                                nc.vector.tensor_tensor(
                                    out=wt[:, 0:nt * NS],
                                    in0=amt[:, aoff + NS:
                                            aoff + (nt + 1) * NS],
                                    in1=l1g[:, 0:nt * NS], op=Alu.mult)
                            return vt, wt

                        def emit_folds(t0, nt, vt, wt):
                            for j in range(nt):
                                t = t0 + j
                                nc.tensor.matmul(
                                    ale_ps[:, 0:NS], idI[:],
                                    vt[:, j * NS:(j + 1) * NS],
                                    start=(t == 0), stop=(t == T - 1),
                                    skip_group_check=True)
                                nc.tensor.matmul(
                                    alt_ps[:, 0:NS], idI[:],
                                    wt[:, j * NS:(j + 1) * NS],
                                    start=(t == 0), stop=(t == T - 1),
                                    skip_group_check=True)

                        # software pipeline: PE stream is
                        #   [pi g][pi g+1][folds g][pi g+2][folds g+1]...
                        # so pi matmuls never queue behind folds waiting on
                        # the DVE products of the previous group.
                        pending = None
                        pi_prev = emit_pi(*GRPS[0])
                        for gi in range(len(GRPS)):
                            t0, nt = GRPS[gi]
                            if gi + 1 < len(GRPS):
                                pi_next = emit_pi(*GRPS[gi + 1])
                            if pending is not None:
                                emit_folds(*pending)
                            vt, wt = emit_mults(t0, nt, pi_prev)
                            pending = (t0, nt, vt, wt)
                            if gi + 1 < len(GRPS):
                                pi_prev = pi_next
                        emit_folds(*pending)

                        # ---- per-chunk tail: S1 reduce + ce to SBUF ----
                        nc.vector.tensor_reduce(
                            out=dacc[:, c:c + 1], in_=ale_ps[:, 0:NS],
                            axis=mybir.AxisListType.X, op=Alu.add)
                        # ce = Ale - Alt (= L1 at event time); stage Alt in
                        # SBUF first (DVE has a single PSUM read port)
                        alt_sb = cpx.tile([128, NS], fp32, tag="alt_sb", bufs=1)
                        nc.vector.tensor_copy(alt_sb[:], alt_ps[:, 0:NS])
                        nc.vector.tensor_tensor(out=ceC[c][:],
                                                in0=ale_ps[:, 0:NS],
                                                in1=alt_sb[:],
                                                op=Alu.subtract)

                # ---- batched finals (both chunks): exps together, then
                # lns together, so the ACT table set switches only twice ----
                Xc = [res.tile([128, NS], fp32, tag=f"X{c}", name=f"X{c}")
                      for c in range(DC)]
                for c in range(DC):
                    nc.scalar.activation(Xc[c][:], ceC[c][:], Act.Exp)
                gts = []
                for c in range(DC):
                    yec = yeT[:, c * NS:(c + 1) * NS]
                    gt_ = cpx.tile([128, NS], fp32, tag=f"g{c}", bufs=1)
                    nc.vector.tensor_tensor(out=gt_[:], in0=Xc[c][:],
                                            in1=yec, op=Alu.add)
                    nc.vector.tensor_scalar(out=gt_[:], in0=gt_[:],
                                            scalar1=-1.0, scalar2=2.0,
                                            op0=Alu.mult, op1=Alu.add)
                    nc.vector.tensor_scalar(out=gt_[:], in0=gt_[:],
                                            scalar1=1e-9, scalar2=None,
                                            op0=Alu.max)
                    gts.append(gt_)
                lns = []
                for c in range(DC):
                    lnG = cpx.tile([128, NS], fp32, tag=f"lnG{c}", bufs=1)
                    nc.scalar.activation(lnG[:], gts[c][:], Act.Ln)
                    lns.append(lnG)
                for c in range(DC):
                    yec = yeT[:, c * NS:(c + 1) * NS]
                    w = cpx.tile([128, NS], fp32, tag=f"w{c}", bufs=1)
                    nc.vector.tensor_tensor(out=w[:], in0=lns[c][:],
                                            in1=ceC[c][:],
                                            op=Alu.subtract)
                    nc.vector.scalar_tensor_tensor(
                        out=w[:], in0=yec, scalar=1.0, in1=w[:],
                        op0=Alu.mult, op1=Alu.mult,
                        accum_out=dacc[:, 108 + c:109 + c])

            nc.sync.dma_start(o_dacc[:], dacc[:])

    if not nc.is_finalized():
        nc.finalize()
    return nc


def _prep_inputs(lambda_, phi, gamma, G, Y, logit_prev_t, event_times):
    lam = np.asarray(lambda_, dtype=np.float32)
    phi = np.asarray(phi, dtype=np.float32)
    gamma = np.asarray(gamma, dtype=np.float32)
    G = np.asarray(G, dtype=np.float32)
    ef = np.asarray(event_times)

    # phi in (t,k)-packed layout [52,32,256] -> [128, 13*256]
    arrp = np.zeros((T, KP, D), np.float32)
    arrp[:, :K, :] = phi.transpose(2, 0, 1)
    phi32 = np.ascontiguousarray(
        arrp.reshape(NBLK, 128, D).transpose(1, 0, 2).reshape(128, NBLK * D)
    ).astype(BF16)

    # fused phi gram input: [phi_row | lp_row] pairs
    prows = phi.reshape(K * D, T)
    lp_rows = np.tile(np.asarray(logit_prev_t, np.float32), (K, 1))
    pair = np.concatenate([prows, lp_rows], axis=1)          # [5120, 104]
    phig2 = np.ascontiguousarray(
        pair.reshape(40, 128, 2 * T).transpose(1, 0, 2).reshape(128, 40 * 2 * T)
    ).astype(BF16)

    gam16 = gamma.astype(BF16)
    tgrid = np.arange(T + 1, dtype=np.float32)
    idmat = np.eye(128, dtype=np.float32).astype(BF16)

    in_maps = []
    for c in range(M):
        sl = slice(c * NS, (c + 1) * NS)
        lam_c = lam[sl]                       # [500, 20, 52]
        arr = np.full((T, KP, NS), -1e4, np.float32)
        arr[:, :K, :] = lam_c.transpose(2, 1, 0)
        lam32 = np.ascontiguousarray(
            arr.reshape(NBLK, 128, NS).transpose(1, 0, 2)
            .reshape(128, NBLK * NS)).astype(BF16)

        efc = ef[sl].astype(np.float32)       # [500, 256]
        efp = efc.T.reshape(DC, 128, NS)      # [2, 128, 500] (chunk, d, n)
        efT = np.ascontiguousarray(
            efp.transpose(1, 0, 2).reshape(128, DC * NS)).astype(BF16)
        ye = np.take_along_axis(np.asarray(Y[sl], np.float32),
                                ef[sl][:, :, None].astype(np.int64),
                                axis=2)[:, :, 0]
        yeT = np.ascontiguousarray(
            ye.T.reshape(DC, 128, NS).transpose(1, 0, 2).reshape(128, DC * NS)
        ).astype(BF16)

        # lam rows with a mean hole: [125, (4*K slots) x 53]
        lamr = lam_c.reshape(4, 125, K, T).transpose(1, 0, 2, 3)  # [125,4,20,52]
        lamh = np.zeros((125, 4, K, T + 1), np.float32)
        lamh[:, :, :, :T] = lamr
        lamg = np.ascontiguousarray(
            lamh.reshape(125, 4 * K * (T + 1))).astype(BF16)
        gtg = np.concatenate([G[sl].T.astype(BF16), gam16], axis=1)

        im = dict(
            lam32=lam32, phi32=phi32, efT=efT, yeT=yeT, lamg=lamg,
            gtg=np.ascontiguousarray(gtg), phig2=phig2, idmat=idmat,
        )
        import os as _os
        if _os.environ.get("KPAGED", "0") != "1":
            # (t<=e) mask, t-major per chunk, T+1 slices: [128,(chunk,t,n)]
            efr = efp.transpose(1, 0, 2)              # [128, chunk, n]
            am_full = (efr[:, :, None, :] >= tgrid[None, None, :, None])
            im["amask"] = np.ascontiguousarray(
                am_full.reshape(128, DC * (T + 1) * NS)).astype(BF16)
        in_maps.append(im)
    return in_maps


def kernel(lambda_, phi, gamma, G, Y, logit_prev_t, event_times):
    from concourse.bass_utils import run_bass_kernel_spmd

    if "nc" not in _COMPILED:
        _COMPILED["nc"] = _build_nc()
    nc = _COMPILED["nc"]

    in_maps = _prep_inputs(lambda_, phi, gamma, G, Y, logit_prev_t, event_times)
    res = run_bass_kernel_spmd(nc, in_maps, list(range(M)))

    data_sum = 0.0
    q_lam = 0.0
    for c in range(M):
        r = res.results[c]
        data_sum += float(r["o_dacc"].astype(np.float64).sum())
        g = r["o_glam"].astype(np.float64)
        A = g[0:T, 0:T]
        b = g[T, 0:T].reshape(T, 1)
        m2 = g[T, T]
        ones = np.ones((T, 1))
        S = A - b @ ones.T - ones @ b.T + m2
        q_lam += float((_KINV_LAM * S).sum())
    gp = res.results[0]["o_gphi"].astype(np.float64)
    Ap = gp[0:T, 0:T]
    Bp = gp[T:2 * T, 0:T]
    Cp = gp[T:2 * T, T:2 * T]
    Sp = Ap - Bp - Bp.T + Cp
    q_phi = float((_KINV_PHI * Sp).sum())

    loss = -data_sum / N + 0.5 * q_lam / N + 0.5 * q_phi / D
    return np.array(loss, dtype=np.float32)


# revision 49
# speedup vs baseline: 1.0533x; 1.0533x over previous
import numpy as np
import ml_dtypes

# Problem constants (hardcoded; kernel.py must be self-contained)
N, D, T, K, P = 4000, 256, 52, 20, 100
M = 8            # cores
NS = N // M      # 500 patients per core
KP = 32          # K padded to 32 so each t-group stays inside one partition tile
NBLK = (T * KP) // 128   # 13 blocks of 128 (t,k) rows
DC = 2           # d-chunks of 128
# main-loop t-groups per chunk: 17 groups of 3 plus one group of 1 (PSUM banks:
# pi4 3 banks x2 bufs + ce 1 bank = 7 of 8)
GRPS = [(3 * i, 3) for i in range(17)] + [(51, 1)]
NG = len(GRPS)

BF16 = ml_dtypes.bfloat16


def _make_kernel_mat(length_scale):
    t = np.arange(T, dtype=np.float32)
    sq = (t[None, :] - t[:, None]) ** 2
    Kmat = np.exp(-0.5 * sq / np.float32(length_scale) ** 2).astype(np.float32)
    jitter = 1e-4
    eye = np.eye(T, dtype=np.float32)
    while True:
        if np.linalg.cond(Kmat + jitter * eye) < 1e4:
            break
        jitter *= 2
        if jitter > 0.1:
            break
    return (Kmat + jitter * eye).astype(np.float32)


_KINV_LAM = np.linalg.inv(_make_kernel_mat(T / 4).astype(np.float64))
_KINV_PHI = np.linalg.inv(_make_kernel_mat(T / 3).astype(np.float64))

_COMPILED = {}


def _build_nc():
    import os
    import concourse.bass as bass
    import concourse.mybir as mybir
    from concourse import bacc, tile

    use_rcp = os.environ.get("KRCP", "1") == "1"
    use_sig = os.environ.get("KSIG", "1") == "1"
    use_paged = os.environ.get("KPAGED", "0") == "1"

    fp32 = mybir.dt.float32
    bf16 = mybir.dt.bfloat16
    Alu = mybir.AluOpType
    Act = mybir.ActivationFunctionType

    nc = bacc.Bacc(None, target_bir_lowering=False)

    # ---- DRAM inputs (host-prepacked layouts) ----
    lam32_d = nc.dram_tensor("lam32", [128, NBLK * NS], bf16, kind="ExternalInput")
    phi32_d = nc.dram_tensor("phi32", [128, NBLK * D], bf16, kind="ExternalInput")
    efT_d = nc.dram_tensor("efT", [128, DC * NS], bf16, kind="ExternalInput")
    yeT_d = nc.dram_tensor("yeT", [128, DC * NS], bf16, kind="ExternalInput")
    # lam rows interleaved with one-hole-per-k for the device-written mean
    lamg_d = nc.dram_tensor("lamg", [125, 4 * K * (T + 1)], bf16,
                            kind="ExternalInput")
    gtg_d = nc.dram_tensor("gtg", [P, NS + K], bf16, kind="ExternalInput")
    # [phi_row | lp_row] pairs for the fused phi gram
    phig2_d = nc.dram_tensor("phig2", [128, 40 * 2 * T], bf16,
                             kind="ExternalInput")
    # identity for the PE event-fold
    id_d = nc.dram_tensor("idmat", [128, 128], bf16, kind="ExternalInput")
    if not use_paged:
        # (t <= e) mask, t-major per chunk, padded to T+1 slices (slice T=0)
        am_d = nc.dram_tensor("amask", [128, DC * (T + 1) * NS], bf16,
                              kind="ExternalInput")

    # ---- DRAM outputs ----
    o_dacc = nc.dram_tensor("o_dacc", [128, 112], fp32, kind="ExternalOutput")
    o_glam = nc.dram_tensor("o_glam", [T + 1, T + 1], fp32, kind="ExternalOutput")
    o_gphi = nc.dram_tensor("o_gphi", [2 * T, 2 * T], fp32, kind="ExternalOutput")

    with tile.TileContext(nc) as tc:
        with (
            tc.tile_pool(name="res", bufs=1) as res,
            tc.tile_pool(name="scr", bufs=4) as scr,
            tc.tile_pool(name="cpx", bufs=2) as cpx,
        ):
            theta = res.tile([128, NBLK * NS], bf16, tag="theta")
            phibar = res.tile([128, NBLK * D], bf16, tag="phibar")
            ones32 = res.tile([128, KP], bf16, tag="ones32")
            nc.vector.memset(ones32[:], 1.0)
            dacc = res.tile([128, 112], fp32, tag="dacc")
            nc.vector.memset(dacc[:], 0.0)

            with (
                tc.tile_pool(name="setup", bufs=1) as setup,
                tc.tile_pool(name="soft", bufs=1) as soft,
            ):
                # critical-path DMAs first: lambda + phi feed the softmax /
                # pi-matmul chain that gates the whole main loop
                lam32 = soft.tile([128, NBLK * NS], bf16, tag="lam32")
                nc.sync.dma_start(lam32[:], lam32_d[:])
                phi32 = setup.tile([128, NBLK * D], bf16, tag="phi32")
                nc.sync.dma_start(phi32[:], phi32_d[:])
                # chunk-0 mask prefetch
                h1 = 28 * NS
                TP1 = T + 1
                if not use_paged:
                    amc0 = res.tile([128, h1], bf16, tag="amc", bufs=2,
                                    name="amc0")
                    nc.sync.dma_start(amc0[:], am_d[:, 0:h1])
                    amc20 = res.tile([128, 26 * NS], bf16, tag="amc2",
                                     bufs=2, name="amc20")
                    nc.sync.dma_start(amc20[:], am_d[:, 27 * NS:TP1 * NS])
                efT = res.tile([128, DC * NS], bf16, tag="efT")
                nc.sync.dma_start(efT[:], efT_d[:])
                yeT = res.tile([128, DC * NS], bf16, tag="yeT")
                nc.sync.dma_start(yeT[:], yeT_d[:])
                idI = res.tile([128, 128], bf16, tag="idI")
                nc.sync.dma_start(idI[:], id_d[:])

                # e32 = exp(lambda) in quarters: R matmuls of early blocks
                # start as soon as their slice is ready
                e32 = soft.tile([128, NBLK * NS], bf16, tag="e32")
                EQ = (3, 4, 3, 3)
                q0 = 0
                for nq in EQ:
                    nc.scalar.activation(
                        e32[:, q0 * NS:(q0 + nq) * NS],
                        lam32[:, q0 * NS:(q0 + nq) * NS], Act.Exp)
                    q0 += nq
                # phibar = 1 - sigmoid(phi) = sigmoid(-phi); chunk-0 d-cols
                # first (they gate the first pi matmuls)
                phv = phibar[:, :].rearrange("p (b d) -> p b d", d=D)
                piv = phi32[:, :].rearrange("p (b d) -> p b d", d=D)
                for cc in range(DC):
                    dsl = slice(cc * 128, (cc + 1) * 128)
                    if use_sig:
                        nc.scalar.activation(phv[:, :, dsl], piv[:, :, dsl],
                                             Act.Sigmoid, scale=-1.0)
                    else:
                        nc.scalar.activation(phv[:, :, dsl], piv[:, :, dsl],
                                             Act.Sigmoid)
                if not use_sig:
                    nc.vector.tensor_scalar(out=phibar[:], in0=phibar[:],
                                            scalar1=-1.0, scalar2=1.0,
                                            op0=Alu.mult, op1=Alu.add)

                # ===== theta = softmax(lambda): e * recip(R) — BEFORE the
                # GP grams so the PE unblocks the main loop asap =====
                rinv16 = soft.tile([128, NBLK * NS], bf16, tag="rinv16")
                with tc.tile_pool(name="rrp", bufs=4,
                                  space=bass.MemorySpace.PSUM) as rrp:
                    for blk in range(NBLK):
                        rrep = rrp.tile([128, 512], fp32, tag="rrep")
                        for pg in range(4):
                            nc.tensor.matmul(
                                rrep[pg * 32:(pg + 1) * 32, 0:NS],
                                ones32[pg * 32:(pg + 1) * 32, :],
                                e32[pg * 32:pg * 32 + KP,
                                    blk * NS:(blk + 1) * NS],
                                skip_group_check=True,
                                tile_position=(pg * 32, pg * 32))
                        if use_rcp:
                            rfp = scr.tile([128, NS], fp32, tag="rfp")
                            nc.vector.reciprocal_approx_fast(
                                rfp[:], rrep[:, 0:NS])
                        else:
                            rsb = scr.tile([128, NS], fp32, tag="rsb")
                            nc.scalar.activation(rsb[:], rrep[:, 0:NS],
                                                 Act.Copy)
                            with nc.allow_low_precision("softmax recip bf16"):
                                nc.vector.reciprocal(
                                    rinv16[:, blk * NS:(blk + 1) * NS],
                                    rsb[:])
                        # per-blk mult so pi matmuls of early groups can
                        # start before the whole softmax finishes
                        nc.vector.tensor_tensor(
                            out=theta[:, blk * NS:(blk + 1) * NS],
                            in0=e32[:, blk * NS:(blk + 1) * NS],
                            in1=(rfp[:] if use_rcp else
                                 rinv16[:, blk * NS:(blk + 1) * NS]),
                            op=Alu.mult)

            # ===== GP phase: mean, then fused grams (PE work that is
            # off the critical path; its tiles reuse the softmax space) =====
            with tc.tile_pool(name="gpsu", bufs=1) as setup:
                lamg = setup.tile([125, 4 * K * (T + 1)], bf16, tag="lamg")
                nc.sync.dma_start(lamg[:], lamg_d[:])
                gtg = setup.tile([P, NS + K], bf16, tag="gtg")
                nc.sync.dma_start(gtg[:], gtg_d[:])
                phig2 = setup.tile([128, 40 * 2 * T], bf16, tag="phig2")
                nc.sync.dma_start(phig2[:], phig2_d[:])
                with tc.tile_pool(name="gps", bufs=1,
                                  space=bass.MemorySpace.PSUM) as gps:
                    mean_ps = gps.tile([125, 512], fp32, tag="mean_ps")
                    for b in range(4):
                        nc.tensor.matmul(mean_ps[:, 0:K],
                                         gtg[:, b * 125:(b + 1) * 125],
                                         gtg[:, NS:NS + K])
                        # write mean into the per-(b,k) hole at slot offset 52
                        hole = lamg[:, :].rearrange(
                            "p (s w) -> p s w", w=T + 1)[:, b * K:(b + 1) * K,
                                                         T:T + 1]
                        nc.vector.tensor_copy(hole,
                                              mean_ps[:, 0:K].unsqueeze(2))

                    glam_ps = gps.tile([T + 1, 512], fp32, tag="glam_ps")
                    nmm = 4 * K
                    for i in range(nmm):
                        v = lamg[:, i * (T + 1):(i + 1) * (T + 1)]
                        nc.tensor.matmul(glam_ps[:, 0:T + 1], v, v,
                                         start=(i == 0), stop=(i == nmm - 1),
                                         skip_group_check=True)
                    gphi_ps = gps.tile([2 * T, 512], fp32, tag="gphi_ps")
                    for i in range(40):
                        v = phig2[:, i * 2 * T:(i + 1) * 2 * T]
                        nc.tensor.matmul(gphi_ps[:, 0:2 * T], v, v,
                                         start=(i == 0), stop=(i == 39),
                                         skip_group_check=True)
                    gout = setup.tile([2 * T, 2 * T + T + 1], fp32, tag="gout")
                    nc.vector.tensor_copy(gout[0:T + 1, 0:T + 1],
                                          glam_ps[0:T + 1, 0:T + 1])
                    nc.vector.tensor_copy(gout[:, T + 1:T + 1 + 2 * T],
                                          gphi_ps[:, 0:2 * T])
                    nc.sync.dma_start(o_glam[:], gout[0:T + 1, 0:T + 1])
                    nc.sync.dma_start(o_gphi[:], gout[:, T + 1:T + 1 + 2 * T])



            # ===== data-loss main loop =====
            # Per chunk: matmul pi4 = 1-pi, scalar-evict Ln -> L1f, then two
            # 2x TT mult passes against the (t<=e) mask and its t+1-shifted
            # view; PE identity-folds both products over t into two PSUM
            # banks: Ale = sum_{t<=e} L1 and Alt = sum_{t<e} L1 (per
            # element). S1 = reduce(Ale); ce = Ale - Alt = L1 at event.
            TP1 = T + 1
            with tc.tile_pool(name="big", bufs=1) as big:
                ceC = [res.tile([128, NS], bf16, tag=f"ce{c}",
                                name=f"ce{c}") for c in range(DC)]
                for c in range(DC):
                    efc = efT[:, c * NS:(c + 1) * NS]
                    if not use_paged:
                        # halves split at the t=27 group boundary; +1 slice
                        # overlap so shifted (w) views stay in-tile.
                        # chunk 0 was prefetched during setup.
                        h1 = 28 * NS
                        if c == 0:
                            amc, amc2 = amc0, amc20
                        else:
                            amc = res.tile([128, h1], bf16, tag="amc",
                                           bufs=2, name="amc1")
                            amc2 = res.tile([128, 26 * NS], bf16,
                                            tag="amc2", bufs=2, name="amc21")
                            nc.sync.dma_start(
                                amc[:],
                                am_d[:, c * TP1 * NS:c * TP1 * NS + h1])
                            nc.sync.dma_start(
                                amc2[:],
                                am_d[:, c * TP1 * NS + 27 * NS:
                                     (c + 1) * TP1 * NS])

                    with (
                        tc.tile_pool(name="pi4p", bufs=2,
                                     space=bass.MemorySpace.PSUM) as pi4p,
                        tc.tile_pool(name="cep", bufs=1,
                                     space=bass.MemorySpace.PSUM) as cep,
                        tc.tile_pool(name="vwp", bufs=3) as vwp,
                        tc.tile_pool(name="l1p", bufs=3) as l1p,
                    ):
                        ale_ps = cep.tile([128, 512], fp32, tag="ale")
                        alt_ps = cep.tile([128, 512], fp32, tag="alt")

                        def emit_pi(t0, nt):
                            pi4 = pi4p.tile([128, 3 * 512], fp32, tag="pi4")
                            for j in range(nt):
                                t = t0 + j
                                blk, prow = t // 4, 32 * (t % 4)
                                nc.tensor.matmul(
                                    pi4[:, j * 512:j * 512 + NS],
                                    phibar[prow:prow + KP,
                                           blk * D + c * 128:
                                           blk * D + c * 128 + 128],
                                    theta[prow:prow + KP,
                                          blk * NS:(blk + 1) * NS],
                                    skip_group_check=True,
                                    tile_position=(prow, 0))
                            return pi4

                        def emit_mults(t0, nt, pi4):
                            pi4v = pi4[:, :].rearrange(
                                "p (t q) -> p t q", q=512)[:, 0:nt, 0:NS]
                            l1g = l1p.tile([128, 3 * NS], bf16, tag="l1g")
                            nc.scalar.activation(
                                l1g[:, 0:nt * NS].rearrange(
                                    "p (t n) -> p t n", t=nt),
                                pi4v, Act.Ln)
                            vt = vwp.tile([128, 3 * NS], bf16, tag="v")
                            wt = vwp.tile([128, 3 * NS], bf16, tag="w")
                            if use_paged:
                                l1v = l1g[:, 0:nt * NS].rearrange(
                                    "p (t n) -> p t n", t=nt)
                                mo = efc.unsqueeze(1).broadcast_to(
                                    [128, nt, NS])
                                nc.vector.tensor_paged_mask(
                                    out=vt[:, 0:nt * NS].rearrange(
                                        "p (t n) -> p t n", t=nt),
                                    in_=l1v, partition_indices=float(t0 - 1),
                                    partition_step=1.0, mask_offsets=mo)
                                nc.vector.tensor_paged_mask(
                                    out=wt[:, 0:nt * NS].rearrange(
                                        "p (t n) -> p t n", t=nt),
                                    in_=l1v, partition_indices=float(t0),
                                    partition_step=1.0, mask_offsets=mo)
                            else:
                                if t0 >= 27:
                                    amt, aoff = amc2, (t0 - 27) * NS
                                else:
                                    amt, aoff = amc, t0 * NS
                                nc.vector.tensor_tensor(
                                    out=vt[:, 0:nt * NS],
                                    in0=amt[:, aoff:aoff + nt * NS],
                                    in1=l1g[:, 0:nt * NS], op=Alu.mult)
                                nc.vector.tensor_tensor(
                                    out=wt[:, 0:nt * NS],
                                    in0=amt[:, aoff + NS:
                                            aoff + (nt + 1) * NS],
                                    in1=l1g[:, 0:nt * NS], op=Alu.mult)
                            return vt, wt

                        def emit_folds(t0, nt, vt, wt):
                            for j in range(nt):
                                t = t0 + j
                                nc.tensor.matmul(
                                    ale_ps[:, 0:NS], idI[:],
                                    vt[:, j * NS:(j + 1) * NS],
                                    start=(t == 0), stop=(t == T - 1),
                                    skip_group_check=True)
                                nc.tensor.matmul(
                                    alt_ps[:, 0:NS], idI[:],
                                    wt[:, j * NS:(j + 1) * NS],
                                    start=(t == 0), stop=(t == T - 1),
                                    skip_group_check=True)

                        # software pipeline: PE stream is
                        #   [pi g][pi g+1][folds g][pi g+2][folds g+1]...
                        # so pi matmuls never queue behind folds waiting on
                        # the DVE products of the previous group.
                        pending = None
                        pi_prev = emit_pi(*GRPS[0])
                        for gi in range(len(GRPS)):
                            t0, nt = GRPS[gi]
                            if gi + 1 < len(GRPS):
                                pi_next = emit_pi(*GRPS[gi + 1])
                            if pending is not None:
                                emit_folds(*pending)
                            vt, wt = emit_mults(t0, nt, pi_prev)
                            pending = (t0, nt, vt, wt)
                            if gi + 1 < len(GRPS):
                                pi_prev = pi_next
                        emit_folds(*pending)

                        # ---- per-chunk tail: S1 reduce + ce to SBUF ----
                        nc.vector.tensor_reduce(
                            out=dacc[:, c:c + 1], in_=ale_ps[:, 0:NS],
                            axis=mybir.AxisListType.X, op=Alu.add)
                        # ce = Ale - Alt (= L1 at event time); stage Alt in
                        # SBUF first (DVE has a single PSUM read port)
                        alt_sb = cpx.tile([128, NS], fp32, tag="alt_sb", bufs=1)
                        nc.vector.tensor_copy(alt_sb[:], alt_ps[:, 0:NS])
                        nc.vector.tensor_tensor(out=ceC[c][:],
                                                in0=ale_ps[:, 0:NS],
                                                in1=alt_sb[:],
                                                op=Alu.subtract)

                # ---- batched finals (both chunks): exps together, then
                # lns together, so the ACT table set switches only twice ----
                Xc = [res.tile([128, NS], fp32, tag=f"X{c}", name=f"X{c}")
                      for c in range(DC)]
                for c in range(DC):
                    nc.scalar.activation(Xc[c][:], ceC[c][:], Act.Exp)
                gts = []
                for c in range(DC):
                    yec = yeT[:, c * NS:(c + 1) * NS]
                    gt_ = cpx.tile([128, NS], fp32, tag=f"g{c}", bufs=1)
                    nc.vector.tensor_tensor(out=gt_[:], in0=Xc[c][:],
                                            in1=yec, op=Alu.add)
                    nc.vector.tensor_scalar(out=gt_[:], in0=gt_[:],
                                            scalar1=-1.0, scalar2=2.0,
                                            op0=Alu.mult, op1=Alu.add)
                    nc.vector.tensor_scalar(out=gt_[:], in0=gt_[:],
                                            scalar1=1e-9, scalar2=None,
                                            op0=Alu.max)
                    gts.append(gt_)
                lns = []
                for c in range(DC):
                    lnG = cpx.tile([128, NS], fp32, tag=f"lnG{c}", bufs=1)
                    nc.scalar.activation(lnG[:], gts[c][:], Act.Ln)
                    lns.append(lnG)
                for c in range(DC):
                    yec = yeT[:, c * NS:(c + 1) * NS]
                    w = cpx.tile([128, NS], fp32, tag=f"w{c}", bufs=1)
                    nc.vector.tensor_tensor(out=w[:], in0=lns[c][:],
                                            in1=ceC[c][:],
                                            op=Alu.subtract)
                    nc.vector.scalar_tensor_tensor(
                        out=w[:], in0=yec, scalar=1.0, in1=w[:],
                        op0=Alu.mult, op1=Alu.mult,
                        accum_out=dacc[:, 108 + c:109 + c])

            nc.sync.dma_start(o_dacc[:], dacc[:])

    if not nc.is_finalized():
        nc.finalize()
    return nc


def _prep_inputs(lambda_, phi, gamma, G, Y, logit_prev_t, event_times):
    lam = np.asarray(lambda_, dtype=np.float32)
    phi = np.asarray(phi, dtype=np.float32)
    gamma = np.asarray(gamma, dtype=np.float32)
    G = np.asarray(G, dtype=np.float32)
    ef = np.asarray(event_times)

    # phi in (t,k)-packed layout [52,32,256] -> [128, 13*256]
    arrp = np.zeros((T, KP, D), np.float32)
    arrp[:, :K, :] = phi.transpose(2, 0, 1)
    phi32 = np.ascontiguousarray(
        arrp.reshape(NBLK, 128, D).transpose(1, 0, 2).reshape(128, NBLK * D)
    ).astype(BF16)

    # fused phi gram input: [phi_row | lp_row] pairs
    prows = phi.reshape(K * D, T)
    lp_rows = np.tile(np.asarray(logit_prev_t, np.float32), (K, 1))
    pair = np.concatenate([prows, lp_rows], axis=1)          # [5120, 104]
    phig2 = np.ascontiguousarray(
        pair.reshape(40, 128, 2 * T).transpose(1, 0, 2).reshape(128, 40 * 2 * T)
    ).astype(BF16)

    gam16 = gamma.astype(BF16)
    tgrid = np.arange(T + 1, dtype=np.float32)
    idmat = np.eye(128, dtype=np.float32).astype(BF16)

    in_maps = []
    for c in range(M):
        sl = slice(c * NS, (c + 1) * NS)
        lam_c = lam[sl]                       # [500, 20, 52]
        arr = np.full((T, KP, NS), -1e4, np.float32)
        arr[:, :K, :] = lam_c.transpose(2, 1, 0)
        lam32 = np.ascontiguousarray(
            arr.reshape(NBLK, 128, NS).transpose(1, 0, 2)
            .reshape(128, NBLK * NS)).astype(BF16)

        efc = ef[sl].astype(np.float32)       # [500, 256]
        efp = efc.T.reshape(DC, 128, NS)      # [2, 128, 500] (chunk, d, n)
        efT = np.ascontiguousarray(
            efp.transpose(1, 0, 2).reshape(128, DC * NS)).astype(BF16)
        ye = np.take_along_axis(np.asarray(Y[sl], np.float32),
                                ef[sl][:, :, None].astype(np.int64),
                                axis=2)[:, :, 0]
        yeT = np.ascontiguousarray(
            ye.T.reshape(DC, 128, NS).transpose(1, 0, 2).reshape(128, DC * NS)
        ).astype(BF16)

        # lam rows with a mean hole: [125, (4*K slots) x 53]
        lamr = lam_c.reshape(4, 125, K, T).transpose(1, 0, 2, 3)  # [125,4,20,52]
        lamh = np.zeros((125, 4, K, T + 1), np.float32)
        lamh[:, :, :, :T] = lamr
        lamg = np.ascontiguousarray(
            lamh.reshape(125, 4 * K * (T + 1))).astype(BF16)
        gtg = np.concatenate([G[sl].T.astype(BF16), gam16], axis=1)

        im = dict(
            lam32=lam32, phi32=phi32, efT=efT, yeT=yeT, lamg=lamg,
            gtg=np.ascontiguousarray(gtg), phig2=phig2, idmat=idmat,
        )
        import os as _os
        if _os.environ.get("KPAGED", "0") != "1":
            # (t<=e) mask, t-major per chunk, T+1 slices: [128,(chunk,t,n)]
            efr = efp.transpose(1, 0, 2)              # [128, chunk, n]
            am_full = (efr[:, :, None, :] >= tgrid[None, None, :, None])
            im["amask"] = np.ascontiguousarray(
                am_full.reshape(128, DC * (T + 1) * NS)).astype(BF16)
        in_maps.append(im)
    return in_maps


def kernel(lambda_, phi, gamma, G, Y, logit_prev_t, event_times):
    from concourse.bass_utils import run_bass_kernel_spmd

    if "nc" not in _COMPILED:
        _COMPILED["nc"] = _build_nc()
    nc = _COMPILED["nc"]

    in_maps = _prep_inputs(lambda_, phi, gamma, G, Y, logit_prev_t, event_times)
    res = run_bass_kernel_spmd(nc, in_maps, list(range(M)))

    data_sum = 0.0
    q_lam = 0.0
    for c in range(M):
        r = res.results[c]
        data_sum += float(r["o_dacc"].astype(np.float64).sum())
        g = r["o_glam"].astype(np.float64)
        A = g[0:T, 0:T]
        b = g[T, 0:T].reshape(T, 1)
        m2 = g[T, T]
        ones = np.ones((T, 1))
        S = A - b @ ones.T - ones @ b.T + m2
        q_lam += float((_KINV_LAM * S).sum())
    gp = res.results[0]["o_gphi"].astype(np.float64)
    Ap = gp[0:T, 0:T]
    Bp = gp[T:2 * T, 0:T]
    Cp = gp[T:2 * T, T:2 * T]
    Sp = Ap - Bp - Bp.T + Cp
    q_phi = float((_KINV_PHI * Sp).sum())

    loss = -data_sum / N + 0.5 * q_lam / N + 0.5 * q_phi / D
    return np.array(loss, dtype=np.float32)
